# revision 8
# baseline (speedup 1.0000x reference)
"""Multi-head attention block for Trainium2, 8-core data-parallel SPMD.

Computes, per batch element b (one NeuronCore each):
    qkv = x @ w_qkv ; q,k,v split into 16 heads of dim 64
    attn = softmax(q @ k^T / sqrt(64)) ; out = (attn @ v) @ w_out + b_out

Strategy (per core):
  phase 0: transpose x -> xT (c-major) via PE transposes
  phase 1: qT, kT = w_qkv^T @ x (c-major), v natural layout; v is written
           strided into v_aug tiles with a ones-column appended per head so
           the second attention matmul also produces softmax row-sums.
  phase 2: per head: s^T chunks = kT^T @ qT on PE (fp32r), exp on ACT with
           the 1/8 scale folded in, o^T_aug accumulated over k chunks.
           softmax normalization deferred: o^T /= rowsum via DVE reciprocal
           + K=1 ones-matmul partition-broadcast.
  phase 3: out = o^T^T @ w_out + ones x b_out (bias added by the PE).
All matmul-feeding tiles are declared float32r (full PE rate; the producing
DVE/ACT/DMA instructions emit the FP32r rounding the BIR verifier requires).
"""

import sys

if "/opt/trn_rl_repo" not in sys.path:
    sys.path.insert(0, "/opt/trn_rl_repo")

import numpy as np

B = 8
N = 1024  # sequence length
C = 1024  # model dim
H = 16  # heads
D = 64  # head dim
P = 128  # partitions
NT = N // P  # seq chunks
CT = C // P  # channel chunks
HP = H // 2  # head pairs
SCALE = D ** -0.5
HF = C // 512  # free-dim halves per 1024 row

_CACHE = {}


def _build_program():
    from concourse import bacc, mybir
    import concourse.tile as tile
    from concourse.masks import make_identity

    f32 = mybir.dt.float32
    f32r = mybir.dt.float32r
    Exp = mybir.ActivationFunctionType.Exp

    nc = bacc.Bacc("TRN2", target_bir_lowering=False, debug=False)
    x_d = nc.declare_dram_parameter("x", [N, C], f32, isOutput=False)
    wqkv_d = nc.declare_dram_parameter("w_qkv", [C, 3 * C], f32r, isOutput=False)
    wout_d = nc.declare_dram_parameter("w_out", [C, C], f32r, isOutput=False)
    bout_d = nc.declare_dram_parameter("b_out", [1, C], f32r, isOutput=False)
    out_d = nc.declare_dram_parameter("out", [N, C], f32, isOutput=True)

    with tile.TileContext(nc) as tc:
        with (
            tc.tile_pool(name="consts", bufs=1) as consts,
            tc.tile_pool(name="xTo", bufs=CT) as xT_pool,
            tc.tile_pool(name="qkT", bufs=2 * CT) as qkT_pool,
            tc.tile_pool(name="vaug", bufs=NT) as vaug_pool,
        ):
            identity = consts.tile([P, P], f32, name="identity", tag="identity")
            make_identity(nc, identity)
            # memset can't emit f32r (ISA check) — stage in f32, round via copy
            ones_f32 = consts.tile([P, P], f32, name="ones_f32", tag="ones_f32")
            nc.vector.memset(ones_f32, 1.0)
            ones = consts.tile([1, P], f32r, name="ones", tag="ones")
            nc.vector.tensor_copy(ones[0:1, :], ones_f32[0:1, :])
            b_row = consts.tile([1, C], f32r, name="b_row", tag="b_row")
            nc.sync.dma_start(out=b_row[0:1, :], in_=bout_d[0:1, :])

            xT = [
                xT_pool.tile([P, N], f32r, name=f"xT{i}", tag="xTo") for i in range(CT)
            ]
            qT = [
                qkT_pool.tile([P, N], f32r, name=f"qT{i}", tag="qkT") for i in range(CT)
            ]
            kT = [
                qkT_pool.tile([P, N], f32r, name=f"kT{i}", tag="qkT") for i in range(CT)
            ]
            vaug = [
                vaug_pool.tile([P, H * (D + 1)], f32r, name=f"vaug{i}", tag="vaug")
                for i in range(NT)
            ]

            # ---------------- phase 0 + 1: xT, then qT/kT/v_aug ----------------
            with (
                tc.tile_pool(name="ps01", bufs=1, space="PSUM") as ps01,
                tc.tile_pool(name="xin", bufs=2) as xin_pool,
                tc.tile_pool(name="wv", bufs=2) as wv_pool,
                tc.tile_pool(name="wqk", bufs=4) as wqk_pool,
            ):
                # phase 0: transpose x into xT
                for si in range(NT):
                    xin = xin_pool.tile([P, C], f32, name=f"xin{si}", tag="xin")
                    nc.sync.dma_start(out=xin[:, :], in_=x_d[si * P : (si + 1) * P, :])
                    tr_ps = ps01.tile([P, C], f32, name=f"tr{si}", tag="mm", bufs=4)
                    for ci in range(CT):
                        nc.tensor.transpose(
                            tr_ps[:, ci * P : (ci + 1) * P],
                            xin[:, ci * P : (ci + 1) * P],
                            identity,
                        )
                    for ci in range(CT):
                        nc.vector.tensor_copy(
                            xT[ci][:, si * P : (si + 1) * P],
                            tr_ps[:, ci * P : (ci + 1) * P],
                        )

                # phase 1B: v in natural layout -> v_aug (ones col per head)
                # seq-chunk groups of 3 to respect PSUM capacity; w_v rows are
                # re-streamed once per group.
                for sc0 in range(0, NT, 3):
                    scs = range(sc0, min(sc0 + 3, NT))
                    vps = {}
                    for sc in scs:
                        vps[sc] = ps01.tile(
                            [P, C], f32, name=f"vps{sc}", tag="mm", bufs=4
                        )
                    for ci in range(CT):
                        wv = wv_pool.tile([P, C], f32r, name=f"wv{sc0}_{ci}", tag="wv")
                        nc.sync.dma_start(
                            out=wv[:, :],
                            in_=wqkv_d[ci * P : (ci + 1) * P, 2 * C : 3 * C],
                        )
                        for sc in scs:
                            for hf in range(HF):
                                sl = slice(hf * 512, hf * 512 + 512)
                                nc.tensor.matmul(
                                    vps[sc][:, sl],
                                    xT[ci][:, sc * P : (sc + 1) * P],
                                    wv[:, sl],
                                    start=(ci == 0),
                                    stop=(ci == CT - 1),
                                )
                    for sc in scs:
                        # ones columns (one per head), then strided v copy
                        va3 = vaug[sc].rearrange("p (h u) -> p h u", u=D + 1)
                        nc.vector.tensor_copy(
                            va3[:, :, D : D + 1],
                            ones_f32[:, 0:H].rearrange("p (h u) -> p h u", u=1),
                        )
                        nc.vector.tensor_copy(
                            va3[:, :, 0:D],
                            vps[sc].rearrange("p (h u) -> p h u", u=D),
                        )

                # phase 1A: qT and kT, c_out chunk pairs (g -> qT[g], kT[g])
                for g in range(CT):
                    q_ps = ps01.tile([P, C], f32, name=f"qps{g}", tag="mm", bufs=4)
                    k_ps = ps01.tile([P, C], f32, name=f"kps{g}", tag="mm", bufs=4)
                    for ci in range(CT):
                        wq = wqk_pool.tile([P, P], f32r, name=f"wq{g}_{ci}", tag="wqk")
                        nc.sync.dma_start(
                            out=wq[:, :],
                            in_=wqkv_d[ci * P : (ci + 1) * P, g * P : (g + 1) * P],
                        )
                        wk = wqk_pool.tile([P, P], f32r, name=f"wk{g}_{ci}", tag="wqk")
                        nc.sync.dma_start(
                            out=wk[:, :],
                            in_=wqkv_d[
                                ci * P : (ci + 1) * P, C + g * P : C + (g + 1) * P
                            ],
                        )
                        st = dict(start=(ci == 0), stop=(ci == CT - 1))
                        for hf in range(HF):
                            sl = slice(hf * 512, hf * 512 + 512)
                            nc.tensor.matmul(
                                q_ps[:, sl], wq[:, :], xT[ci][:, sl], **st
                            )
                            nc.tensor.matmul(
                                k_ps[:, sl], wk[:, :], xT[ci][:, sl], **st
                            )
                    nc.vector.tensor_copy(qT[g][:, :], q_ps[:, :])
                    nc.vector.tensor_copy(kT[g][:, :], k_ps[:, :])

            # ---------------- phase 2: attention per head pair ----------------
            with (
                tc.tile_pool(name="ps2", bufs=1, space="PSUM") as ps2,
                tc.tile_pool(name="pT", bufs=4) as pT_pool,
                tc.tile_pool(name="recip", bufs=2) as recip_pool,
                tc.tile_pool(name="bcs", bufs=2) as bcs_pool,
            ):
                oT = [
                    xT_pool.tile([P, N], f32r, name=f"oT{i}", tag="xTo")
                    for i in range(CT)
                ]
                for t in range(HP):
                    accs = []
                    for j in range(2):
                        acc = ps2.tile(
                            [P, N], f32, name=f"acc{2 * t + j}", tag="acc", bufs=2
                        )
                        accs.append(acc)
                    for kc in range(NT):
                        ss = []
                        for j in range(2):
                            s_ps = ps2.tile(
                                [P, N], f32, name=f"s{2 * t + j}_{kc}", tag="sT", bufs=2
                            )
                            ss.append(s_ps)
                        # interleave the two heads (rows 0-63 / 64-127) so the
                        # K=64 matmuls overlap in distinct PE row groups
                        for hf in range(HF):
                            sl = slice(hf * 512, hf * 512 + 512)
                            for j in range(2):
                                row0 = D * j
                                nc.tensor.matmul(
                                    ss[j][:, sl],
                                    kT[t][row0 : row0 + D, kc * P : (kc + 1) * P],
                                    qT[t][row0 : row0 + D, sl],
                                    start=True,
                                    stop=True,
                                )
                        st = dict(start=(kc == 0), stop=(kc == NT - 1))
                        for j in range(2):
                            h = 2 * t + j
                            pt = pT_pool.tile(
                                [P, N], f32r, name=f"pt{h}_{kc}", tag="pT"
                            )
                            nc.scalar.activation(
                                out=pt[:, :], in_=ss[j][:, :], func=Exp, scale=SCALE
                            )
                            for hf in range(HF):
                                sl = slice(hf * 512, hf * 512 + 512)
                                nc.tensor.matmul(
                                    accs[j][0 : D + 1, sl],
                                    vaug[kc][:, h * (D + 1) : (h + 1) * (D + 1)],
                                    pt[:, sl],
                                    **st,
                                )
                    # normalize: o^T[d, q] *= 1 / rowsum[q]
                    for j in range(2):
                        h = 2 * t + j
                        row0 = D * j
                        rc = recip_pool.tile([1, N], f32r, name=f"rc{h}", tag="recip")
                        with nc.allow_low_precision(
                            reason="softmax norm reciprocal rounded to f32r "
                            "for the PE broadcast matmul"
                        ):
                            nc.vector.reciprocal(rc[0:1, :], accs[j][D : D + 1, :])
                        bc = ps2.tile([P, N], f32, name=f"bc{h}", tag="sT", bufs=2)
                        for hf in range(HF):
                            sl = slice(hf * 512, hf * 512 + 512)
                            nc.tensor.matmul(
                                bc[0:D, sl],
                                ones[0:1, 0:D],
                                rc[0:1, sl],
                                start=True,
                                stop=True,
                            )
                        # DVE reads at most one PSUM operand: stage bc in SBUF
                        bcs = bcs_pool.tile([D, N], f32, name=f"bcs{h}", tag="bcs")
                        nc.vector.tensor_copy(bcs[0:D, :], bc[0:D, :])
                        nc.vector.tensor_mul(
                            oT[t][row0 : row0 + D, :],
                            accs[j][0:D, :],
                            bcs[0:D, :],
                        )

            # ---------------- phase 3: out = o @ w_out + b ----------------
            with (
                tc.tile_pool(name="ps3", bufs=1, space="PSUM") as ps3,
                tc.tile_pool(name="wo", bufs=CT) as wo_pool,
                tc.tile_pool(name="outp", bufs=3) as outp_pool,
            ):
                wos = []
                for ci in range(CT):
                    wo = wo_pool.tile([P, C], f32r, name=f"wo{ci}", tag="wo")
                    nc.sync.dma_start(
                        out=wo[:, :], in_=wout_d[ci * P : (ci + 1) * P, :]
                    )
                    wos.append(wo)
                for sc in range(NT):
                    o_ps = ps3.tile([P, C], f32, name=f"ops{sc}", tag="mm3", bufs=3)
                    for ci in range(CT):
                        for hf in range(HF):
                            sl = slice(hf * 512, hf * 512 + 512)
                            nc.tensor.matmul(
                                o_ps[:, sl],
                                oT[ci][:, sc * P : (sc + 1) * P],
                                wos[ci][:, sl],
                                start=(ci == 0),
                                stop=False,
                            )
                    for hf in range(HF):
                        sl = slice(hf * 512, hf * 512 + 512)
                        nc.tensor.matmul(
                            o_ps[:, sl],
                            ones[0:1, 0:P],
                            b_row[0:1, sl],
                            start=False,
                            stop=True,
                        )
                    ot = outp_pool.tile([P, C], f32, name=f"ot{sc}", tag="outp")
                    nc.vector.tensor_copy(ot[:, :], o_ps[:, :])
                    nc.sync.dma_start(
                        out=out_d[sc * P : (sc + 1) * P, :], in_=ot[:, :]
                    )

    nc.compile()
    return nc


def _get_program():
    if "nc" not in _CACHE:
        _CACHE["nc"] = _build_program()
    return _CACHE["nc"]


def kernel(x, w_qkv, w_out, b_out):
    from concourse.bass_utils import run_bass_kernel_spmd

    nc = _get_program()
    x = np.ascontiguousarray(np.asarray(x, dtype=np.float32))
    w_qkv = np.ascontiguousarray(np.asarray(w_qkv, dtype=np.float32))
    w_out = np.ascontiguousarray(np.asarray(w_out, dtype=np.float32))
    b_row = np.ascontiguousarray(np.asarray(b_out, dtype=np.float32).reshape(1, C))
    in_maps = [
        {"x": x[i], "w_qkv": w_qkv, "w_out": w_out, "b_out": b_row} for i in range(B)
    ]
    res = run_bass_kernel_spmd(nc, in_maps, core_ids=list(range(B))).results
    return np.stack([res[i]["out"] for i in range(B)], axis=0)
